# revision 2
# baseline (speedup 1.0000x reference)
"""Multi-head causal attention on 8 TRN2 NeuronCores — one head per core.

v3: chunk-pair-major schedule. For each 1024-query chunk-pair cp, stream
scores+exp over key tiles jt=0..8cp+7 (ScalarE stays fed from ~3us), then
accumulate O^T for its two 512-chunks with dual-fp8 DoubleRow matmuls over
key-tile pairs and emit the output projection. P^T lives in small per-cp
fp8 tiles. exp has a -1 bias (cancels in softmax) to fit fp8e4.
"""

import numpy as np
import ml_dtypes

import concourse.bass as bass
import concourse.mybir as mybir
import concourse.tile as tile
from concourse import bacc
from concourse.bass_utils import run_bass_kernel_spmd

BF16 = mybir.dt.bfloat16
F8 = mybir.dt.float8e4
F32 = mybir.dt.float32

S = 4096
D_IN = 512
D_K = 64
D_V = 64
D_OUT = 512
H = 8
NJT = S // 128    # 32 key tiles
NPT = NJT // 2    # 16 key-tile pairs
NCH = S // 512    # 8 query chunks
NCP = 4           # chunk-pairs (1024 queries each)
NCK = D_IN // 128  # contraction chunks for projections
VW = 80           # V' half stride (fp8 dual-row needs 16B-mult k-tile step)

_CACHE = {}


def _emit(nc, tc, ctx_pools):
    import contextlib

    xT_d = nc.dram_tensor("xT", [D_IN, S], BF16, kind="ExternalInput").ap()
    wq_d = nc.dram_tensor("wq", [D_IN, 128], BF16, kind="ExternalInput").ap()
    wk_d = nc.dram_tensor("wk", [D_IN, 128], BF16, kind="ExternalInput").ap()
    wv_d = nc.dram_tensor("wv", [D_IN, D_V], BF16, kind="ExternalInput").ap()
    wo_d = nc.dram_tensor("wo", [D_V, D_OUT], BF16, kind="ExternalInput").ap()
    mask_d = nc.dram_tensor("mask", [128, 128], F8, kind="ExternalInput").ap()
    iden_d = nc.dram_tensor("iden", [128, 128], BF16, kind="ExternalInput").ap()
    out_d = nc.dram_tensor("out", [S, D_OUT], F32, kind="ExternalOutput").ap()

    Exp = mybir.ActivationFunctionType.Exp

    with contextlib.ExitStack() as ctx:
        const = ctx.enter_context(tc.tile_pool(name="const", bufs=1))
        persist = ctx.enter_context(tc.tile_pool(name="persist", bufs=1))
        small = ctx.enter_context(tc.tile_pool(name="small", bufs=3))
        outp = ctx.enter_context(tc.tile_pool(name="outp", bufs=3))

        # ---- constants ----
        wq_sb = const.tile([128, NCK * 128], BF16)
        wk_sb = const.tile([128, NCK * 128], BF16)
        wv_sb = const.tile([128, NCK * D_V], BF16)
        wo_sb = const.tile([D_V, D_OUT], BF16)
        mask_sb = const.tile([128, 128], F8)
        iden_sb = const.tile([128, 128], BF16)
        bias_sb = const.tile([128, 1], F32)
        nc.vector.memset(bias_sb, -1.0)
        for c in range(NCK):
            rows = slice(c * 128, (c + 1) * 128)
            nc.gpsimd.dma_start(out=wq_sb[:, c * 128:(c + 1) * 128], in_=wq_d[rows, :])
            nc.gpsimd.dma_start(out=wk_sb[:, c * 128:(c + 1) * 128], in_=wk_d[rows, :])
            nc.gpsimd.dma_start(out=wv_sb[:, c * D_V:(c + 1) * D_V], in_=wv_d[rows, :])
        nc.gpsimd.dma_start(out=wo_sb, in_=wo_d)
        nc.gpsimd.dma_start(out=mask_sb, in_=mask_d)
        nc.gpsimd.dma_start(out=iden_sb, in_=iden_d)

        # persistent activations
        qt = persist.tile([128, S], BF16)   # Q^T duplicated in both halves
        kt = persist.tile([128, S], BF16)   # K^T duplicated in both halves
        vpt = persist.tile([64, S], BF16)   # V^T
        vp = persist.tile([128, NPT * 2 * VW], F8)  # V' pairs
        nc.vector.memset(vp, 0.0)
        nc.vector.memset(
            vp.rearrange("p (t j w) -> p t j w", j=2, w=VW)[:, :, :, 64], 1.0)

        def vp_pair(t):
            return vp[:, t * 2 * VW:(t + 1) * 2 * VW].rearrange(
                "p (j w) -> p j w", w=VW)

        # x^T arrives in three waves of SEPARATE tiles so the first
        # projections depend only on wave A (tile deps are tile-granular)
        WAVES = [(0, 1024), (1024, 2560), (2560, S)]
        xts = {}  # (wave, c) -> tile
        for w, (a, b) in enumerate(WAVES):
            for c in range(NCK):
                xt = persist.tile([128, b - a], BF16, tag=f"xt{w}_{c}",
                                  name=f"xt{w}_{c}")
                eng = nc.sync if (w != 1) else nc.gpsimd
                eng.dma_start(out=xt, in_=xT_d[c * 128:(c + 1) * 128, a:b])
                xts[(w, c)] = xt

        def proj(w_sb, wid, dest, st, pool, tag="psA"):
            a = st * 512
            w = 0 if a < 1024 else (1 if a < 2560 else 2)
            off = a - WAVES[w][0]
            ps = pool.tile([wid, 512], F32, tag=tag, name=f"ps{wid}_{st}")
            for c in range(NCK):
                nc.tensor.matmul(
                    ps,
                    lhsT=w_sb[:, c * wid:(c + 1) * wid],
                    rhs=xts[(w, c)][:, off:off + 512],
                    start=(c == 0),
                    stop=(c == NCK - 1),
                )
            nc.vector.tensor_copy(dest[:, a:a + 512], ps)

        # stage A: exactly what cp=0's first scores need
        with tc.tile_pool(name="psA", bufs=4, space="PSUM") as psA:
            proj(wq_sb, 128, qt, 0, psA)
            proj(wq_sb, 128, qt, 1, psA)
            proj(wk_sb, 128, kt, 0, psA)

        # ---- chunk-pair-major fused pass ----
        from collections import deque
        pending = deque()

        def drain(n):
            if len(pending) > 48:
                n += 4
            for _ in range(n):
                if not pending:
                    return
                pending.popleft()()

        def vp_transpose(jt):
            def go():
                pst = psAcc.tile([128, 64], BF16, tag="bank", name=f"pst{jt}")
                nc.tensor.transpose(
                    pst,
                    vpt[:, jt * 128:(jt + 1) * 128],
                    iden_sb[0:64, 0:64],
                )
                t, j = jt // 2, jt % 2
                nc.vector.tensor_copy(
                    vp[:, t * 2 * VW + j * VW:t * 2 * VW + j * VW + 64], pst)
            return go

        def filler_proj(w_sb, wid, dest, st):
            def go():
                proj(w_sb, wid, dest, st, psAcc, tag="bank")
            return go

        accs = {}
        ptps = {}

        def enqueue_ot(c, ts_, cp):
            if c not in accs:
                accs[c] = psAcc.tile([65, 512], F32, tag="bank",
                                     name=f"acc{c}")
            acc = accs[c]

            def ot_mm(tp):
                view = ptps[tp]  # capture now: the slot is rebound next cp

                def go():
                    lo = max(c * 512, tp * 256)
                    hi = (c + 1) * 512
                    nc.tensor.matmul(
                        acc[:, lo - c * 512:hi - c * 512],
                        lhsT=vp_pair(tp)[:, :, 0:65],
                        rhs=view[:, :, lo - cp * 1024:hi - cp * 1024],
                        start=(tp == 0),
                        stop=(tp == 2 * c + 1),
                        perf_mode=mybir.MatmulPerfMode.DoubleRow,
                    )
                return go

            for tp in ts_:
                pending.append(ot_mm(tp))

        def enqueue_fin(c):
            acc = accs[c]

            def evac():
                ot_bf = small.tile([65, 512], BF16, tag="otbf")
                nc.vector.tensor_copy(ot_bf, acc)
                se_bf = small.tile([128, 4], BF16, tag="se_bf")
                for ib in range(4):
                    nc.gpsimd.dma_start(
                        out=se_bf[:, ib:ib + 1],
                        in_=ot_bf[64:65, ib * 128:(ib + 1) * 128],
                    ) if c % 2 == 0 else nc.sync.dma_start(
                        out=se_bf[:, ib:ib + 1],
                        in_=ot_bf[64:65, ib * 128:(ib + 1) * 128],
                    )
                rcols = small.tile([128, 4], F32, tag="rcols")
                nc.vector.reciprocal(rcols, se_bf)

                def out_proj(ib):
                    def go():
                        po = psAcc.tile([128, 512], F32, tag="bank",
                                        name=f"po{c}_{ib}")
                        nc.tensor.matmul(
                            po,
                            lhsT=ot_bf[0:64, ib * 128:(ib + 1) * 128],
                            rhs=wo_sb,
                            start=True,
                            stop=True,
                        )
                        ob = outp.tile([128, 512], F32, tag="ob")
                        nc.vector.tensor_scalar_mul(
                            ob, po, rcols[:, ib:ib + 1])
                        nc.sync.dma_start(
                            out=out_d[c * 512 + ib * 128:
                                      c * 512 + (ib + 1) * 128, :],
                            in_=ob,
                        )
                    return go

                for ib in range(4):
                    pending.append(out_proj(ib))

            pending.append(evac)

        with tc.tile_pool(name="psB", bufs=2, space="PSUM") as psB, \
             tc.tile_pool(name="psAcc", bufs=4, space="PSUM") as psAcc, \
             tc.tile_pool(name="pt", bufs=2) as pt_pool:
            # fillers, ordered by first use (FIFO drain during exp waits)
            pending.append(filler_proj(wk_sb, 128, kt, 1))
            for st in (0, 1):
                pending.append(filler_proj(wv_sb, 64, vpt, st))
                for j2 in range(4 * st, 4 * st + 4):
                    pending.append(vp_transpose(j2))
            for g in (1, 2, 3):
                pending.append(filler_proj(wq_sb, 128, qt, 2 * g))
                pending.append(filler_proj(wq_sb, 128, qt, 2 * g + 1))
                pending.append(filler_proj(wk_sb, 128, kt, 2 * g))
                pending.append(filler_proj(wk_sb, 128, kt, 2 * g + 1))
                for st in (2 * g, 2 * g + 1):
                    pending.append(filler_proj(wv_sb, 64, vpt, st))
                    for j2 in range(4 * st, 4 * st + 4):
                        pending.append(vp_transpose(j2))

            for cp in range(NCP):
                q0 = cp * 1024
                for jt in range(8 * cp + 8):
                    i0 = jt * 128
                    t, half = jt // 2, jt % 2
                    if half == 0:
                        ptf = pt_pool.tile([128, 2048], F8, tag=f"pt{t}",
                                           name=f"pt{cp}_{t}")
                        ptps[t] = ptf.rearrange("p (j n) -> p j n", j=2)
                    ptp = ptps[t]
                    if half == 1 and i0 > q0:
                        # garbage head of this diagonal pair's second half
                        nc.vector.memset(
                            ptp[:, 1, i0 - 128 - q0:i0 - q0], 0.0)
                    glo = max(q0, i0)
                    ps = psB.tile([128, 1024], F32, tag="psB")
                    for c in (2 * cp, 2 * cp + 1):
                        lo = max(c * 512, i0)
                        hi = (c + 1) * 512
                        if lo < hi:
                            nc.tensor.matmul(
                                ps[:, lo - q0:hi - q0],
                                lhsT=kt[:, jt * 128:(jt + 1) * 128],
                                rhs=qt[:, lo:hi],
                                start=True,
                                stop=True,
                            )
                    nc.scalar.activation(
                        ptp[:, half, glo - q0:1024],
                        ps[:, glo - q0:1024],
                        Exp,
                        scale=0.0625,  # 1/sqrt(64) / 2 (duplicated contraction)
                        bias=bias_sb,  # -1: shift exp into fp8e4 range
                    )
                    drain(3)
                    if i0 >= q0:
                        # causal mask on the diagonal 128x128 block
                        dsl = ptp[:, half, i0 - q0:i0 - q0 + 128]
                        nc.vector.tensor_mul(dsl, dsl, mask_sb)
                    if jt == 8 * cp + 1:
                        enqueue_ot(2 * cp, range(0, 4 * cp + 1), cp)
                    elif jt == 8 * cp + 3:
                        enqueue_ot(2 * cp, [4 * cp + 1], cp)
                        enqueue_fin(2 * cp)
                    elif jt == 8 * cp + 5:
                        enqueue_ot(2 * cp + 1, range(0, 4 * cp + 3), cp)
                    elif jt == 8 * cp + 7:
                        enqueue_ot(2 * cp + 1, [4 * cp + 3], cp)
                        enqueue_fin(2 * cp + 1)
            while pending:
                drain(8)


def _build():
    if "nc" in _CACHE:
        return _CACHE["nc"]
    nc = bacc.Bacc("TRN2", target_bir_lowering=False, debug=False)
    with tile.TileContext(nc) as tc:
        _emit(nc, tc, None)
    nc.compile()
    _CACHE["nc"] = nc
    return nc


def build_in_maps(x, W_q, W_k, W_v, W_o):
    bf = ml_dtypes.bfloat16
    f8 = ml_dtypes.float8_e4m3fn
    xT = np.ascontiguousarray(x.reshape(S, D_IN).T).astype(bf)
    mask = np.triu(np.ones((128, 128), np.float32)).astype(f8)
    iden = np.eye(128, dtype=np.float32).astype(bf)
    in_maps = []
    for h in range(H):
        wq2 = np.concatenate([W_q[h], W_q[h]], axis=1)  # [512, 128]
        wk2 = np.concatenate([W_k[h], W_k[h]], axis=1)
        in_maps.append({
            "xT": xT,
            "wq": np.ascontiguousarray(wq2).astype(bf),
            "wk": np.ascontiguousarray(wk2).astype(bf),
            "wv": np.ascontiguousarray(W_v[h]).astype(bf),
            "wo": np.ascontiguousarray(W_o[h]).astype(bf),
            "mask": mask,
            "iden": iden,
        })
    return in_maps


def kernel(x, W_q, W_k, W_v, W_o):
    nc = _build()
    in_maps = build_in_maps(x, W_q, W_k, W_v, W_o)
    res = run_bass_kernel_spmd(nc, in_maps, core_ids=list(range(H)))
    out = np.zeros((S, D_OUT), np.float32)
    for h in range(H):
        out += res.results[h]["out"].astype(np.float32)
    return out[None]
